# revision 1
# baseline (speedup 1.0000x reference)
"""MoE (T=2048 H=2048 I=1408 E=16 top-2) on 8 trn2 NeuronCores.

Strategy (expert-parallel, per the sharding hint):
  - Router (gate linear + top-2 sigmoid + renorm) computed on host in f64
    (matches the f32 reference's top-k selections with margin to spare).
  - Tokens are dispatched host-side: each expert e gets its routed tokens
    gathered into a fixed-capacity batch of C slots (C = max expert load
    rounded up to 64).  Core c owns experts {2c, 2c+1}.
  - Each core runs a Bass/Tile kernel computing, per owned expert:
        gT = Wg_e.T @ X_e.T   uT = Wu_e.T @ X_e.T        (bf16 matmul)
        hT = silu(gT) * uT                                (fp32, cast bf16)
        y  = hT.T @ Wd_e  scaled per-token by the combine weight
    Weights are streamed through SBUF in DMA-friendly slabs; activations
    keep fp32 accumulation in PSUM.
  - Host combines: out[t] = y[slot0(t)] + y[slot1(t)] (weights already
    applied on-device; padded slots carry weight 0 and are never gathered).
"""

import numpy as np
import ml_dtypes

import concourse.bacc as bacc
import concourse.mybir as mybir
import concourse.tile as tile
from concourse.bass_utils import run_bass_kernel_spmd

T = 2048
H = 2048
I = 1408
E = 16
K = 2
NCORES = 8
EPC = E // NCORES  # experts per core
P = 128
HT = H // P        # 16 h-tiles
IT = I // P        # 11 i-tiles
NOUT = 512         # down-proj output tile (one PSUM bank of fp32)
NT = H // NOUT     # 4 n-tiles

BF16 = mybir.dt.bfloat16
F32 = mybir.dt.float32
nbf = ml_dtypes.bfloat16

_kernel_cache: dict[int, tuple] = {}
_weight_cache: dict[tuple, tuple] = {}


def _build(C: int):
    """Build + compile the per-core expert kernel for capacity C (multiple of 64, <=512)."""
    TC = EPC * C
    MT = -(-C // P)  # m-tiles per expert

    nc = bacc.Bacc("TRN2", target_bir_lowering=False, debug=False, num_devices=NCORES)
    xt = nc.dram_tensor("xt", [H, TC], BF16, kind="ExternalInput")
    # weight slabs, laid out host-side to make every DMA fully contiguous:
    #   wgs[e, i, p, h*P + c] = Wg[e, h*P + p, i*P + c]   (column slab of [H, I])
    #   wds[e, i, p, :]       = Wd[e, i*P + p, :]         (row tile of [I, H])
    wgs = nc.dram_tensor("wgs", [EPC, IT, P, H], BF16, kind="ExternalInput")
    wus = nc.dram_tensor("wus", [EPC, IT, P, H], BF16, kind="ExternalInput")
    wds = nc.dram_tensor("wds", [EPC, IT, P, H], BF16, kind="ExternalInput")
    cw = nc.dram_tensor("cw", [P, EPC * MT], F32, kind="ExternalInput")
    yo = nc.dram_tensor("yo", [TC, H], F32, kind="ExternalOutput")

    with tile.TileContext(nc) as tc:
        with (
            tc.tile_pool(name="xt_pool", bufs=1) as xt_pool,
            tc.tile_pool(name="cw_pool", bufs=1) as cw_pool,
            tc.tile_pool(name="wg_pool", bufs=4) as wg_pool,
            tc.tile_pool(name="wu_pool", bufs=4) as wu_pool,
            tc.tile_pool(name="wd_pool", bufs=IT + 3) as wd_pool,
            tc.tile_pool(name="ht_pool", bufs=2 * IT + 1) as ht_pool,
            tc.tile_pool(name="tmp_pool", bufs=2) as tmp_pool,
            tc.tile_pool(name="out_pool", bufs=4) as out_pool,
            tc.tile_pool(name="pg_pool", bufs=2, space="PSUM") as pg_pool,
            tc.tile_pool(name="pu_pool", bufs=2, space="PSUM") as pu_pool,
            tc.tile_pool(name="py_pool", bufs=3, space="PSUM") as py_pool,
        ):
            xt_r = xt.ap().rearrange("(ht p) t -> ht p t", p=P)
            xt_sb = xt_pool.tile([P, HT, TC], BF16)
            for h in range(HT):
                nc.sync.dma_start(xt_sb[:, h, :], xt_r[h])
            cw_sb = cw_pool.tile([P, EPC * MT], F32)
            nc.sync.dma_start(cw_sb[:], cw.ap()[:, :])

            for e in range(EPC):
                tok = slice(e * C, (e + 1) * C)
                # ---- gate/up projections, one I-column-slab at a time ----
                hts = []
                for i in range(IT):
                    wg_t = wg_pool.tile([P, H], BF16, tag="wg")
                    nc.sync.dma_start(wg_t[:], wgs.ap()[e, i])
                    wu_t = wu_pool.tile([P, H], BF16, tag="wu")
                    nc.sync.dma_start(wu_t[:], wus.ap()[e, i])
                    pg = pg_pool.tile([P, C], F32, tag="pg")
                    pu = pu_pool.tile([P, C], F32, tag="pu")
                    for h in range(HT):
                        nc.tensor.matmul(
                            pg[:],
                            wg_t[:, h * P:(h + 1) * P],
                            xt_sb[:, h, tok],
                            start=(h == 0),
                            stop=(h == HT - 1),
                        )
                    for h in range(HT):
                        nc.tensor.matmul(
                            pu[:],
                            wu_t[:, h * P:(h + 1) * P],
                            xt_sb[:, h, tok],
                            start=(h == 0),
                            stop=(h == HT - 1),
                        )
                    tmp = tmp_pool.tile([P, C], F32, tag="tmp")
                    nc.scalar.activation(
                        tmp[:], pg[:], mybir.ActivationFunctionType.Silu
                    )
                    ht_t = ht_pool.tile([P, C], BF16, tag="ht")
                    nc.vector.tensor_tensor(
                        ht_t[:], tmp[:], pu[:], mybir.AluOpType.mult
                    )
                    hts.append(ht_t)
                # ---- down projection + per-token combine weight ----
                wd_tiles = []
                for i in range(IT):
                    wd_t = wd_pool.tile([P, H], BF16, tag="wd")
                    nc.sync.dma_start(wd_t[:], wds.ap()[e, i])
                    wd_tiles.append(wd_t)
                for k in range(MT):
                    m0 = k * P
                    ms = min(P, C - m0)
                    for n in range(NT):
                        py = py_pool.tile([P, NOUT], F32, tag="py")
                        for i in range(IT):
                            nc.tensor.matmul(
                                py[:ms],
                                hts[i][:, m0:m0 + ms],
                                wd_tiles[i][:, n * NOUT:(n + 1) * NOUT],
                                start=(i == 0),
                                stop=(i == IT - 1),
                            )
                        ot = out_pool.tile([P, NOUT], F32, tag="ot")
                        nc.vector.tensor_scalar_mul(
                            ot[:ms], py[:ms], cw_sb[:ms, e * MT + k:e * MT + k + 1]
                        )
                        nc.sync.dma_start(
                            yo.ap()[e * C + m0:e * C + m0 + ms,
                                    n * NOUT:(n + 1) * NOUT],
                            ot[:ms],
                        )

    nc.compile()
    return nc, TC, MT


def _get_kernel(C: int):
    if C not in _kernel_cache:
        _kernel_cache[C] = _build(C)
    return _kernel_cache[C]


def _prep_weights(w_gate_proj, w_up_proj, w_down_proj):
    key = tuple(
        (a.__array_interface__["data"][0], a.shape)
        for a in (w_gate_proj, w_up_proj, w_down_proj)
    )
    if key in _weight_cache:
        return _weight_cache[key]
    wg_bf = np.asarray(w_gate_proj, np.float32).astype(nbf)  # [E, H, I]
    wu_bf = np.asarray(w_up_proj, np.float32).astype(nbf)    # [E, H, I]
    wd_bf = np.asarray(w_down_proj, np.float32).astype(nbf)  # [E, I, H]
    wg_slab = np.ascontiguousarray(
        wg_bf.reshape(E, HT, P, IT, P).transpose(0, 3, 2, 1, 4).reshape(E, IT, P, H)
    )
    wu_slab = np.ascontiguousarray(
        wu_bf.reshape(E, HT, P, IT, P).transpose(0, 3, 2, 1, 4).reshape(E, IT, P, H)
    )
    wd_rows = np.ascontiguousarray(wd_bf.reshape(E, IT, P, H))
    _weight_cache.clear()
    _weight_cache[key] = (wg_slab, wu_slab, wd_rows)
    return _weight_cache[key]


def _route(X, WG):
    """f64 replica of the reference router; returns per-expert dispatch."""
    logits = X.astype(np.float64) @ np.asarray(WG, np.float64)
    scores = 1.0 / (1.0 + np.exp(-logits))
    top2 = np.argsort(-scores, axis=1, kind="stable")[:, :K]
    w = np.take_along_axis(scores, top2, 1)
    wn = (w / w.sum(1, keepdims=True)).astype(np.float32)
    tok_list, w_list = [], []
    for e in range(E):
        hit = top2 == e  # [T, K]
        tok = np.nonzero(hit.any(1))[0]
        kk = hit[tok, 1].astype(np.int64)
        tok_list.append(tok)
        w_list.append(wn[tok, kk])
    return tok_list, w_list


def _run(inputs: dict, trace: bool = False, trace_cores=None):
    X = np.ascontiguousarray(np.asarray(inputs["hidden_states"], np.float32))
    tok_list, w_list = _route(X, inputs["w_gate"])
    counts = np.array([len(t) for t in tok_list])
    C = max(64, int(-(-counts.max() // 64)) * 64)
    if C > 512:
        raise RuntimeError(f"expert load {counts.max()} exceeds supported capacity")
    nc, TC, MT = _get_kernel(C)
    wg_slab, wu_slab, wd_rows = _prep_weights(
        inputs["w_gate_proj"], inputs["w_up_proj"], inputs["w_down_proj"]
    )

    in_maps = []
    for c in range(NCORES):
        slots_tokens = np.zeros(TC, np.int64)
        cw_host = np.zeros((P, EPC * MT), np.float32)
        for j in range(EPC):
            e = c * EPC + j
            n_e = counts[e]
            slots_tokens[j * C:j * C + n_e] = tok_list[e]
            s = np.arange(n_e)
            cw_host[s % P, j * MT + s // P] = w_list[e]
        xt_host = np.ascontiguousarray(X[slots_tokens].T).astype(nbf)
        in_maps.append(
            {
                "xt": xt_host,
                "wgs": wg_slab[c * EPC:(c + 1) * EPC],
                "wus": wu_slab[c * EPC:(c + 1) * EPC],
                "wds": wd_rows[c * EPC:(c + 1) * EPC],
                "cw": cw_host,
            }
        )

    if trace:
        _install_trace_shim()
    res = run_bass_kernel_spmd(
        nc,
        in_maps,
        core_ids=list(range(NCORES)),
        trace=trace,
        trace_cores=trace_cores,
    )

    Y = np.concatenate([res.results[c]["yo"] for c in range(NCORES)], axis=0)
    slot0 = np.full(T, -1, np.int64)
    slot1 = np.full(T, -1, np.int64)
    for e in range(E):
        tok = tok_list[e]
        base = (e // EPC) * TC + (e % EPC) * C
        rows = base + np.arange(counts[e])
        first = slot0[tok] < 0
        slot0[tok[first]] = rows[first]
        slot1[tok[~first]] = rows[~first]
    assert (slot0 >= 0).all() and (slot1 >= 0).all()
    out = Y[slot0] + Y[slot1]
    return out.astype(np.float32), res


def kernel(**inputs) -> np.ndarray:
    out, _ = _run(inputs, trace=False)
    return out


def _install_trace_shim():
    """Make run_bass_kernel_spmd(trace=True) work under axon: register the
    NTFF profile hook that the slim agent image's antenv stub lacks."""
    import sys, types

    if "antenv.axon_hooks" not in sys.modules:
        import antenv

        mod = types.ModuleType("antenv.axon_hooks")
        mod._hook = None
        mod.set_axon_ntff_profile_hook = lambda h: setattr(mod, "_hook", h)
        mod.get_axon_ntff_profile_hook = lambda: mod._hook
        sys.modules["antenv.axon_hooks"] = mod
        antenv.axon_hooks = mod
    if sys.modules["antenv.axon_hooks"].get_axon_ntff_profile_hook() is None:
        from trn_agent_boot.trn_boot import _ntff_profile_via_ctypes

        sys.modules["antenv.axon_hooks"].set_axon_ntff_profile_hook(
            _ntff_profile_via_ctypes("/opt/axon/libaxon_pjrt.so")
        )


# revision 2
# speedup vs baseline: 1.1589x; 1.1589x over previous
"""MoE (T=2048 H=2048 I=1408 E=16 top-2) on 8 trn2 NeuronCores.

Strategy (expert-parallel, per the sharding hint):
  - Router (gate linear + top-2 sigmoid + renorm) computed on host in f64
    (matches the f32 reference's top-k selections with margin to spare).
  - Tokens are dispatched host-side: experts are sorted by load; the 8
    heaviest go in slot 0 (capacity C0 = max load rounded to 16) and the 8
    lightest in slot 1 (capacity C1), one of each per core.
  - Each core runs a Bass/Tile kernel computing, per owned expert:
        gT = Wg_e.T @ X_e.T   uT = Wu_e.T @ X_e.T        (bf16 matmul)
        hT = silu(gT) * uT                                (fp32, cast bf16)
        yT = Wd_e.T(row-tiles) @ hT, scaled by the per-token combine weight
    Weights are streamed through SBUF in DMA-contiguous slabs; accumulation
    stays fp32 in PSUM.  Output is produced transposed ([H, slots]) so every
    matmul runs with full 128-row stationary tiles and N = capacity.
  - Host combines: out[t] = yT[:, slot0(t)] + yT[:, slot1(t)] (weights are
    already applied on-device; padded slots carry weight 0).
"""

import numpy as np
import ml_dtypes

import concourse.bacc as bacc
import concourse.mybir as mybir
import concourse.tile as tile
from concourse.bass_utils import run_bass_kernel_spmd

T = 2048
H = 2048
I = 1408
E = 16
K = 2
NCORES = 8
EPC = E // NCORES  # experts per core (2)
P = 128
HT = H // P        # 16 h-tiles
IT = I // P        # 11 i-tiles

BF16 = mybir.dt.bfloat16
F32 = mybir.dt.float32
nbf = ml_dtypes.bfloat16

_kernel_cache: dict[tuple, tuple] = {}
_weight_cache: dict[tuple, tuple] = {}


def _build(caps: tuple[int, ...]):
    """Build + compile the per-core kernel for slot capacities `caps` (each a
    multiple of 16, <= 512)."""
    TC = sum(caps)
    starts = [sum(caps[:j]) for j in range(EPC)]

    nc = bacc.Bacc("TRN2", target_bir_lowering=False, debug=False, num_devices=NCORES)
    xt = nc.dram_tensor("xt", [H, TC], BF16, kind="ExternalInput")
    # weight slabs, laid out host-side so every DMA is fully contiguous:
    #   wgs[e, i, p, h*P + c] = Wg[e, h*P + p, i*P + c]   (column slab of [H, I])
    #   wds[e, i, p, :]       = Wd[e, i*P + p, :]         (row tile of [I, H])
    wgs = nc.dram_tensor("wgs", [EPC, IT, P, H], BF16, kind="ExternalInput")
    wus = nc.dram_tensor("wus", [EPC, IT, P, H], BF16, kind="ExternalInput")
    wds = nc.dram_tensor("wds", [EPC, IT, P, H], BF16, kind="ExternalInput")
    cw = nc.dram_tensor("cw", [P, TC], F32, kind="ExternalInput")  # replicated rows
    yo = nc.dram_tensor("yo", [H, TC], F32, kind="ExternalOutput")

    with tile.TileContext(nc) as tc:
        with (
            tc.tile_pool(name="xt_pool", bufs=HT) as xt_pool,
            tc.tile_pool(name="cw_pool", bufs=1) as cw_pool,
            tc.tile_pool(name="wg_pool", bufs=4) as wg_pool,
            tc.tile_pool(name="wu_pool", bufs=4) as wu_pool,
            tc.tile_pool(name="wd_pool", bufs=IT + 3) as wd_pool,
            tc.tile_pool(name="ht_pool", bufs=2 * IT + 1) as ht_pool,
            tc.tile_pool(name="tmp_pool", bufs=2) as tmp_pool,
            tc.tile_pool(name="out_pool", bufs=4) as out_pool,
            tc.tile_pool(name="pg_pool", bufs=2, space="PSUM") as pg_pool,
            tc.tile_pool(name="pu_pool", bufs=2, space="PSUM") as pu_pool,
            tc.tile_pool(name="py_pool", bufs=3, space="PSUM") as py_pool,
        ):
            xt_r = xt.ap().rearrange("(ht p) t -> ht p t", p=P)

            # front-load the first expert's first slabs, then the tokens —
            # the first matmul needs only xt tile 0 + slab (0, 0).
            wg_first = wg_pool.tile([P, H], BF16, tag="wg")
            nc.sync.dma_start(wg_first[:], wgs.ap()[0, 0])
            wu_first = wu_pool.tile([P, H], BF16, tag="wu")
            nc.sync.dma_start(wu_first[:], wus.ap()[0, 0])
            xt_tiles = []
            for h in range(HT):
                xt_t = xt_pool.tile([P, TC], BF16, tag="xt")
                nc.sync.dma_start(xt_t[:], xt_r[h])
                xt_tiles.append(xt_t)
            cw_sb = cw_pool.tile([P, TC], F32)
            nc.sync.dma_start(cw_sb[:], cw.ap()[:, :])

            for e in range(EPC):
                C = caps[e]
                tok = slice(starts[e], starts[e] + C)
                # ---- gate/up projections, one I-column-slab at a time ----
                hts = []
                for i in range(IT):
                    if e == 0 and i == 0:
                        wg_t, wu_t = wg_first, wu_first
                    else:
                        wg_t = wg_pool.tile([P, H], BF16, tag="wg")
                        nc.sync.dma_start(wg_t[:], wgs.ap()[e, i])
                        wu_t = wu_pool.tile([P, H], BF16, tag="wu")
                        nc.sync.dma_start(wu_t[:], wus.ap()[e, i])
                    pg = pg_pool.tile([P, max(caps)], F32, tag="pg")
                    pu = pu_pool.tile([P, max(caps)], F32, tag="pu")
                    for h in range(HT):
                        nc.tensor.matmul(
                            pg[:, :C],
                            wg_t[:, h * P:(h + 1) * P],
                            xt_tiles[h][:, tok],
                            start=(h == 0),
                            stop=(h == HT - 1),
                        )
                    for h in range(HT):
                        nc.tensor.matmul(
                            pu[:, :C],
                            wu_t[:, h * P:(h + 1) * P],
                            xt_tiles[h][:, tok],
                            start=(h == 0),
                            stop=(h == HT - 1),
                        )
                    tmp = tmp_pool.tile([P, max(caps)], F32, tag="tmp")
                    nc.scalar.activation(
                        tmp[:, :C], pg[:, :C], mybir.ActivationFunctionType.Silu
                    )
                    ht_t = ht_pool.tile([P, max(caps)], BF16, tag="ht")
                    nc.vector.tensor_tensor(
                        ht_t[:, :C], tmp[:, :C], pu[:, :C], mybir.AluOpType.mult
                    )
                    hts.append(ht_t)
                # ---- down projection (transposed output) + combine weight ----
                wd_tiles = []
                for i in range(IT):
                    wd_t = wd_pool.tile([P, H], BF16, tag="wd")
                    nc.sync.dma_start(wd_t[:], wds.ap()[e, i])
                    wd_tiles.append(wd_t)
                for h in range(HT):
                    py = py_pool.tile([P, max(caps)], F32, tag="py")
                    for i in range(IT):
                        nc.tensor.matmul(
                            py[:, :C],
                            wd_tiles[i][:, h * P:(h + 1) * P],
                            hts[i][:, :C],
                            start=(i == 0),
                            stop=(i == IT - 1),
                        )
                    ot = out_pool.tile([P, max(caps)], F32, tag="ot")
                    nc.vector.tensor_tensor(
                        ot[:, :C], py[:, :C], cw_sb[:, tok], mybir.AluOpType.mult
                    )
                    # output DMAs ride the SW-DGE (gpsimd) so they never
                    # head-of-line-block the sync queue's weight prefetch.
                    nc.gpsimd.dma_start(
                        yo.ap()[h * P:(h + 1) * P, tok], ot[:, :C]
                    )

    nc.compile()
    return nc, TC, starts


def _get_kernel(caps):
    if caps not in _kernel_cache:
        _kernel_cache[caps] = _build(caps)
    return _kernel_cache[caps]


def _prep_weights(w_gate_proj, w_up_proj, w_down_proj):
    key = tuple(
        (a.__array_interface__["data"][0], a.shape)
        for a in (w_gate_proj, w_up_proj, w_down_proj)
    )
    if key in _weight_cache:
        return _weight_cache[key]
    wg_bf = np.asarray(w_gate_proj, np.float32).astype(nbf)  # [E, H, I]
    wu_bf = np.asarray(w_up_proj, np.float32).astype(nbf)    # [E, H, I]
    wd_bf = np.asarray(w_down_proj, np.float32).astype(nbf)  # [E, I, H]
    wg_slab = np.ascontiguousarray(
        wg_bf.reshape(E, HT, P, IT, P).transpose(0, 3, 2, 1, 4).reshape(E, IT, P, H)
    )
    wu_slab = np.ascontiguousarray(
        wu_bf.reshape(E, HT, P, IT, P).transpose(0, 3, 2, 1, 4).reshape(E, IT, P, H)
    )
    wd_rows = np.ascontiguousarray(wd_bf.reshape(E, IT, P, H))
    _weight_cache.clear()
    _weight_cache[key] = (wg_slab, wu_slab, wd_rows)
    return _weight_cache[key]


def _route(X, WG):
    """f64 replica of the reference router; returns per-expert dispatch."""
    logits = X.astype(np.float64) @ np.asarray(WG, np.float64)
    scores = 1.0 / (1.0 + np.exp(-logits))
    top2 = np.argsort(-scores, axis=1, kind="stable")[:, :K]
    w = np.take_along_axis(scores, top2, 1)
    wn = (w / w.sum(1, keepdims=True)).astype(np.float32)
    tok_list, w_list = [], []
    for e in range(E):
        hit = top2 == e  # [T, K]
        tok = np.nonzero(hit.any(1))[0]
        kk = hit[tok, 1].astype(np.int64)
        tok_list.append(tok)
        w_list.append(wn[tok, kk])
    return tok_list, w_list


def _run(inputs: dict, trace: bool = False, trace_cores=None):
    X = np.ascontiguousarray(np.asarray(inputs["hidden_states"], np.float32))
    tok_list, w_list = _route(X, inputs["w_gate"])
    counts = np.array([len(t) for t in tok_list])

    # slot assignment: heaviest 8 experts in slot 0, lightest 8 in slot 1
    order = np.argsort(-counts, kind="stable")
    slot_exp = [order[:NCORES], order[NCORES:]]  # [slot][core] -> expert
    caps = tuple(
        min(512, max(64, int(-(-counts[slot_exp[j]].max() // 16)) * 16))
        for j in range(EPC)
    )
    if counts.max() > 512:
        raise RuntimeError(f"expert load {counts.max()} exceeds supported capacity")
    nc, TC, starts = _get_kernel(caps)
    wg_slab, wu_slab, wd_rows = _prep_weights(
        inputs["w_gate_proj"], inputs["w_up_proj"], inputs["w_down_proj"]
    )

    in_maps = []
    for c in range(NCORES):
        experts = [int(slot_exp[j][c]) for j in range(EPC)]
        slots_tokens = np.zeros(TC, np.int64)
        cw_row = np.zeros(TC, np.float32)
        for j, e in enumerate(experts):
            n_e = counts[e]
            slots_tokens[starts[j]:starts[j] + n_e] = tok_list[e]
            cw_row[starts[j]:starts[j] + n_e] = w_list[e]
        xt_host = np.ascontiguousarray(X[slots_tokens].T).astype(nbf)
        cw_host = np.ascontiguousarray(np.broadcast_to(cw_row, (P, TC)))
        in_maps.append(
            {
                "xt": xt_host,
                "wgs": np.ascontiguousarray(wg_slab[experts]),
                "wus": np.ascontiguousarray(wu_slab[experts]),
                "wds": np.ascontiguousarray(wd_rows[experts]),
                "cw": cw_host,
            }
        )

    if trace:
        _install_trace_shim()
    res = run_bass_kernel_spmd(
        nc,
        in_maps,
        core_ids=list(range(NCORES)),
        trace=trace,
        trace_cores=trace_cores,
    )

    # combine: out[t] = sum of its two expert outputs (weights already applied)
    Yh = np.concatenate([res.results[c]["yo"] for c in range(NCORES)], axis=1)
    slot0 = np.full(T, -1, np.int64)
    slot1 = np.full(T, -1, np.int64)
    for j in range(EPC):
        for c in range(NCORES):
            e = int(slot_exp[j][c])
            tok = tok_list[e]
            cols = c * TC + starts[j] + np.arange(counts[e])
            first = slot0[tok] < 0
            slot0[tok[first]] = cols[first]
            slot1[tok[~first]] = cols[~first]
    assert (slot0 >= 0).all() and (slot1 >= 0).all()
    out = np.ascontiguousarray((Yh[:, slot0] + Yh[:, slot1]).T)
    return out.astype(np.float32), res


def kernel(**inputs) -> np.ndarray:
    out, _ = _run(inputs, trace=False)
    return out


def _install_trace_shim():
    """Make run_bass_kernel_spmd(trace=True) work under axon: register the
    NTFF profile hook that the slim agent image's antenv stub lacks."""
    import sys, types

    if "antenv.axon_hooks" not in sys.modules:
        import antenv

        mod = types.ModuleType("antenv.axon_hooks")
        mod._hook = None
        mod.set_axon_ntff_profile_hook = lambda h: setattr(mod, "_hook", h)
        mod.get_axon_ntff_profile_hook = lambda: mod._hook
        sys.modules["antenv.axon_hooks"] = mod
        antenv.axon_hooks = mod
    if sys.modules["antenv.axon_hooks"].get_axon_ntff_profile_hook() is None:
        from trn_agent_boot.trn_boot import _ntff_profile_via_ctypes

        sys.modules["antenv.axon_hooks"].set_axon_ntff_profile_hook(
            _ntff_profile_via_ctypes("/opt/axon/libaxon_pjrt.so")
        )
